# revision 14
# baseline (speedup 1.0000x reference)
"""AdapterBank kernel for 8 TRN2 NeuronCores.

Strategy: data-parallel over B=8 (one batch element per core, no
collectives). Each core computes all 5 adapters (1 generalized + 4
specialized) on its h_teacher slice in bf16 matmuls with fp32 PSUM
accumulation, returning [80, 2048] = 5 adapters x [K=16, SD=2048].
The tiny router MLP (0.003% of FLOPs) + top-k gather run on host in
fp32 — router logit gaps are ~4e-4, so low-precision on-device routing
would mis-select experts.

Algebraic folds (exact):
  - scores scale 1/sqrt(HD) folded into qT (host).
  - bk dropped: softmax is invariant to per-row constants.
  - bv folded into ob_eff = bv @ ow + ob (softmax rows sum to 1).
  - LN(x)*lg+lb before wk/wv folded into wk_eff = lg[:,None]*wk (and
    bk_eff absorbed as above); lb contributes lb@wk which is constant
    along rows -> dropped for wk (softmax) and folded into ob for wv.
"""

import sys
sys.path.insert(0, '/opt/trn_rl_repo')

import numpy as np
import ml_dtypes

import concourse.bass as bass
import concourse.mybir as mybir
import concourse.tile as tile
from concourse.bass_utils import run_bass_kernel_spmd
from concourse.masks import make_identity

f32 = mybir.dt.float32
bf16 = mybir.dt.bfloat16
AF = mybir.ActivationFunctionType
ALU = mybir.AluOpType

B, N, TD, SD, BD, K, M, TOPK, H = 8, 1024, 5120, 2048, 1024, 16, 4, 2, 8
HD = SD // H
NA = M + 1          # adapters: [g, s0..s3]
KT_TD = TD // 128   # 40
KT_SD = SD // 128   # 16
KT_BD = BD // 128   # 8
RT = N // 128       # 8 row tiles
EPS = 1e-5

_CACHE = {}


def _split_multi_waits(nc):
    """This container's walrus accepts only one sync-wait per instruction;
    hoist extras into standalone EventSemaphore insts on the same engine."""
    for fn in nc.m.functions:
        for bb in fn.blocks:
            out = []
            for inst in bb.instructions:
                si = inst.sync_info
                if si is not None and len(si.on_wait) > 1:
                    waits = list(si.on_wait)
                    for j, w in enumerate(waits[:-1]):
                        out.append(mybir.InstEventSemaphore(
                            name=f"{inst.name}_w{j}",
                            engine=inst.engine,
                            sync_info=mybir.SyncInfo(on_wait=[w], on_update=[]),
                        ))
                    inst.sync_info = mybir.SyncInfo(
                        on_wait=[waits[-1]], on_update=list(si.on_update))
                out.append(inst)
            bb.instructions[:] = out


def _build(split=True):
    nc = bass.Bass("TRN2", target_bir_lowering=False, debug=False, num_devices=8)

    dm = lambda nm, shp, dt: nc.dram_tensor(nm, shp, dt, kind="ExternalInput").ap()
    hT_d = dm("hT", [2, KT_TD, 128, 512], bf16)
    dw_d = dm("dw", [NA, 2, KT_TD, 128, 512], bf16)
    uw_d = dm("uw", [NA, 2, 128, KT_BD, 1024], bf16)
    wk_d = dm("wk", [NA, 2, 8, 128, KT_SD, 128], bf16)
    wv_d = dm("wv", [NA, 8, 128, KT_SD, 256], bf16)
    ow_d = dm("ow", [NA, 8, 128, KT_SD, 256], bf16)
    wq_d = dm("wq", [NA, 8, 128, KT_SD, 256], bf16)
    qT_d = dm("qT", [128, KT_SD, 80], bf16)
    bq_d = dm("bq", [NA, 128, KT_SD], f32)
    db_d = dm("db", [NA, 128, KT_BD], f32)
    ub_d = dm("ub", [NA, 1, SD], bf16)
    addq_d = dm("addq", [80, SD], bf16)
    pg_d = dm("pg", [80, SD], bf16)
    pb_d = dm("pb", [80, SD], bf16)
    out_d = nc.dram_tensor("out", [80, SD], f32, kind="ExternalOutput").ap()

    from contextlib import ExitStack
    with tile.TileContext(nc) as tc, ExitStack() as es:
        P_ = lambda **kw: es.enter_context(tc.tile_pool(**kw))
        # ---- pools (KB/partition noted) ----
        pW = P_(name="pW", bufs=2)     # w quarters [128,16,512]bf16 16 -> 32
        pWk = P_(name="pWk", bufs=2)   # wk cols [128,16,128]bf16 4 / hT [128,512] -> 8
        pDw = P_(name="pDw", bufs=2)   # dw rows [128,1024]bf16 2 -> 6
        pX1 = P_(name="pX1", bufs=1)   # x1T [128,8,1024]bf16 16
        pUw = P_(name="pUw", bufs=2)   # uw half [128,8,1024]bf16 16 -> 32
        pUb = P_(name="pUb", bufs=1)   # ub [128,2048]f32 8
        pX2 = P_(name="pX2", bufs=2)   # x2pre/x2hat [128,2048]bf16 4 -> 12
        pX2T = P_(name="pX2T", bufs=1) # x2T [128,16,1024]bf16 32
        pKV = P_(name="pKV", bufs=1)   # kpT/vp half [128,8,1024]bf16 16 -> 32
        pQ = P_(name="pQ", bufs=1)     # qT resident 2.5 + qp 4 + qpT 0.5
        pAtt = P_(name="pAtt", bufs=1) # att tiles small
        pCtx = P_(name="pCtx", bufs=1) # ctx [16,2048]bf16 4 + ctxT 0.5
        pOut = P_(name="pOut", bufs=1) # cagg 8 + addq 8 + pg/pb 8 + xh 4
        pSt = P_(name="pSt", bufs=8)  # [128,1] stats
        pB = P_(name="pB", bufs=1)     # db/bq consts

        psum = P_(name="psum", bufs=1, space="PSUM")
        # tag "quad": [128,512]-max tiles, 4 slots x 1 bank
        # tag "wide": [128,2048]-max tiles, 1 slot x 4 banks
        QUAD = dict(tag="quad", bufs=4)
        WIDE = dict(tag="wide", bufs=1)

        # ---- persistent tiles ----
        ident = pB.tile([128, 128], bf16, tag="ident")
        make_identity(nc, ident[:])
        eps_sb = pB.tile([128, 1], f32, tag="eps")
        nc.vector.memset(eps_sb[:], EPS)
        qT_sb = pQ.tile([128, KT_SD, 80], bf16, tag="qT")
        nc.sync.dma_start(qT_sb[:], qT_d[:])

        for a in range(NA):
            # ---------- A: x1T = gelu(h @ dw + db).T  -> [BD, N] ----------
            db_sb = pB.tile([128, KT_BD], f32, tag="db", bufs=2)
            nc.sync.dma_start(db_sb[:], db_d[a])
            x1T = pX1.tile([128, KT_BD, N], bf16, tag="x1T")
            for rh in range(2):
                for ph in range(2):
                    psa = [psum.tile([128, 512], f32, name=f"psa_{a}_{rh}_{ph}_{p}", **QUAD)
                           for p in range(4)]
                    for k in range(KT_TD):
                        ht = pWk.tile([128, 512], bf16, tag="wkcol")
                        nc.sync.dma_start(ht[:], hT_d[rh, k])
                        dwt = pDw.tile([128, 512], bf16, tag="dw")
                        nc.sync.dma_start(dwt[:], dw_d[a, ph, k])
                        for p in range(4):
                            nc.tensor.matmul(psa[p][:], dwt[:, p * 128:(p + 1) * 128],
                                             ht[:], start=(k == 0), stop=(k == KT_TD - 1))
                    for p in range(4):
                        pg_ = ph * 4 + p
                        nc.scalar.activation(x1T[:, pg_, rh * 512:(rh + 1) * 512],
                                             psa[p][:], AF.Gelu,
                                             bias=db_sb[:, pg_:pg_ + 1], scale=1.0)

            # ---------- B+C: x2 = LN(x1 @ uw + ub); x2T ----------
            ub_sb = pUb.tile([128, SD], bf16, tag="ub")
            nc.sync.dma_start(ub_sb[:], ub_d[a].broadcast_to((128, SD)))
            uw_sb = []
            for hf in range(2):
                t = pUw.tile([128, KT_BD, 1024], bf16, tag="uw")
                nc.sync.dma_start(t[:], uw_d[a, hf])
                uw_sb.append(t)
            x2T = pX2T.tile([128, KT_SD, N], bf16, tag="x2T")
            for r in range(RT):
                ps2 = psum.tile([128, SD], f32, **WIDE)
                for hf in range(2):
                    for k in range(KT_BD):
                        lhs = x1T[:, k, r * 128:(r + 1) * 128]
                        for ch in range(2):
                            nc.tensor.matmul(
                                ps2[:, hf * 1024 + ch * 512: hf * 1024 + (ch + 1) * 512],
                                lhs, uw_sb[hf][:, k, ch * 512:(ch + 1) * 512],
                                start=(k == 0), stop=(k == KT_BD - 1))
                # x2pre = psum + ub (bf16), s1 = rowsum
                x2pre = pX2.tile([128, SD], bf16, tag="x2")
                s1 = pSt.tile([128, 1], f32, tag="st")
                nc.vector.scalar_tensor_tensor(x2pre[:], ps2[:], 1.0, ub_sb[:],
                                               ALU.mult, ALU.add, accum_out=s1[:])
                negmu = pSt.tile([128, 1], f32, tag="st")
                nc.vector.tensor_scalar_mul(negmu[:], s1[:], -1.0 / SD)
                # var: Square(x2pre - mu) -> accum vs ; throwaway full out
                x2hat = pX2.tile([128, SD], bf16, tag="x2")
                vs = pSt.tile([128, 1], f32, tag="st")
                nc.scalar.activation(x2hat[:], x2pre[:], AF.Square,
                                     bias=negmu[:], scale=1.0, accum_out=vs[:])
                sd_ = pSt.tile([128, 1], f32, tag="st")
                nc.scalar.activation(sd_[:], vs[:], AF.Sqrt, bias=eps_sb[:],
                                     scale=1.0 / SD)
                rstd = pSt.tile([128, 1], f32, tag="st")
                nc.vector.reciprocal(rstd[:], sd_[:])
                nmr = pSt.tile([128, 1], f32, tag="st")
                nc.vector.scalar_tensor_tensor(nmr[:], negmu[:], 1.0, rstd[:],
                                               ALU.mult, ALU.mult)
                nc.scalar.activation(x2hat[:], x2pre[:], AF.Identity,
                                     bias=nmr[:], scale=rstd[:])
                # C: transpose into x2T
                for s in range(KT_SD):
                    pt = psum.tile([128, 128], bf16, **QUAD)
                    nc.tensor.transpose(pt[:], x2hat[:, s * 128:(s + 1) * 128], ident[:])
                    nc.vector.tensor_copy(x2T[:, s, r * 128:(r + 1) * 128], pt[:])

            # ---------- F: qp = (q @ wq + bq)/16 ; qpT ----------
            bq_sb = pB.tile([128, KT_SD], f32, tag="bq", bufs=2)
            nc.sync.dma_start(bq_sb[:], bq_d[a])
            qp_sb = pQ.tile([16, SD], bf16, tag="qp")
            for q8 in range(8):
                wqt = pW.tile([128, KT_SD, 256], bf16, tag="wquart")
                nc.sync.dma_start(wqt[:], wq_d[a, q8])
                psq = psum.tile([16, 256], f32, name=f"psq_{a}_{q8}", **QUAD)
                for k in range(KT_SD):
                    nc.tensor.matmul(psq[:],
                                     qT_sb[:, k, a * 16:(a + 1) * 16], wqt[:, k, :],
                                     start=(k == 0), stop=(k == KT_SD - 1))
                nc.vector.tensor_copy(qp_sb[:, q8 * 256:(q8 + 1) * 256], psq[:])
            qpT = pQ.tile([128, KT_SD, 16], bf16, tag="qpT")
            for s in range(KT_SD):
                pt = psum.tile([128, 128], bf16, **QUAD)
                nc.tensor.transpose(pt[:, 0:16], qp_sb[:, s * 128:(s + 1) * 128], ident[0:16, 0:16])
                nc.scalar.activation(qpT[:, s, :], pt[:, 0:16], AF.Identity,
                                     bias=bq_sb[:, s:s + 1], scale=1.0)

            ctx_sb = pCtx.tile([16, SD], bf16, tag="ctx")
            for hh in range(2):
                # ---------- D: kpT half ----------
                kpT = pKV.tile([128, 8, N], bf16, tag="kv")
                for p in range(8):
                    wkt = pWk.tile([128, KT_SD, 128], bf16, tag="wkcol")
                    nc.sync.dma_start(wkt[:], wk_d[a, hh, p])
                    pkv = [psum.tile([128, 512], f32, name=f"pkv_{a}_{hh}_{p}_{c}", **QUAD)
                           for c in range(2)]
                    for k in range(KT_SD):
                        for ch in range(2):
                            nc.tensor.matmul(pkv[ch][:],
                                             wkt[:, k, :], x2T[:, k, ch * 512:(ch + 1) * 512],
                                             start=(k == 0), stop=(k == KT_SD - 1))
                    for ch in range(2):
                        nc.vector.tensor_copy(kpT[:, p, ch * 512:(ch + 1) * 512], pkv[ch][:])
                # ---------- G: scores + softmax + attT ----------
                # head hl occupies rows hl*32 .. hl*32+16 (32-aligned starts);
                # unused rows zeroed -> exp(0)=1, harmless.
                att = pAtt.tile([128, N], bf16, tag="att")
                nc.vector.memset(att[:], 0.0)
                for hl in range(4):
                    pss = [psum.tile([16, 512], f32, name=f"pss_{a}_{hh}_{hl}_{c}", **QUAD)
                           for c in range(2)]
                    for j in range(2):
                        st_g = (hh * 4 + hl) * 2 + j
                        for ch in range(2):
                            nc.tensor.matmul(pss[ch][:],
                                             qpT[:, st_g, :],
                                             kpT[:, hl * 2 + j, ch * 512:(ch + 1) * 512],
                                             start=(j == 0), stop=(j == 1))
                    for ch in range(2):
                        nc.vector.tensor_copy(
                            att[hl * 32:hl * 32 + 16, ch * 512:(ch + 1) * 512], pss[ch][:])
                # scores are O(1): exp without max-subtract is safe in f32
                atte = pAtt.tile([128, N], bf16, tag="atte")
                se = pSt.tile([128, 1], f32, tag="st")
                nc.scalar.activation(atte[:], att[:], AF.Exp, accum_out=se[:])
                rs = pSt.tile([128, 1], f32, tag="st")
                nc.vector.reciprocal(rs[:], se[:])
                attn = pAtt.tile([128, N], bf16, tag="attn")
                nc.vector.tensor_scalar_mul(attn[:], atte[:], rs[:])
                attT = pAtt.tile([128, RT, 128], bf16, tag="attT")
                for r in range(RT):
                    pt = psum.tile([128, 128], bf16, name=f"ptt_{a}_{hh}_{r}", **QUAD)
                    nc.tensor.transpose(pt[:], attn[:, r * 128:(r + 1) * 128],
                                        ident[:])
                    nc.vector.tensor_copy(attT[:, r, :], pt[:])
                # ---------- E: vp half ----------
                vp = pKV.tile([128, RT, 1024], bf16, tag="kv")
                for q8 in range(hh * 4, hh * 4 + 4):
                    wvt = pW.tile([128, KT_SD, 256], bf16, tag="wquart")
                    nc.sync.dma_start(wvt[:], wv_d[a, q8])
                    cc = (q8 % 4) * 256
                    for r in range(RT):
                        pv = psum.tile([128, 256], f32, **QUAD)
                        for k in range(KT_SD):
                            nc.tensor.matmul(pv[:], x2T[:, k, r * 128:(r + 1) * 128],
                                             wvt[:, k, :],
                                             start=(k == 0), stop=(k == KT_SD - 1))
                        nc.vector.tensor_copy(vp[:, r, cc:cc + 256], pv[:])
                # ---------- H: ctx half ----------
                pc = [psum.tile([16, 256], f32, name=f"pc_{a}_{hh}_{c}", **QUAD)
                      for c in range(4)]
                for r in range(RT):
                    for hl in range(4):
                        nc.tensor.matmul(pc[hl][:],
                                         attT[:, r, hl * 32:hl * 32 + 16],
                                         vp[:, r, hl * 256:(hl + 1) * 256],
                                         start=(r == 0), stop=(r == RT - 1))
                for c in range(4):
                    nc.vector.tensor_copy(
                        ctx_sb[:, hh * 1024 + c * 256:hh * 1024 + (c + 1) * 256], pc[c][:])

            # ---------- ctxT + I: ctx2 = ctx @ ow (+addq) ----------
            ctxT = pCtx.tile([128, KT_SD, 16], bf16, tag="ctxT")
            for s in range(KT_SD):
                pt = psum.tile([128, 128], bf16, **QUAD)
                nc.tensor.transpose(pt[:, 0:16], ctx_sb[:, s * 128:(s + 1) * 128],
                                    ident[0:16, 0:16])
                nc.vector.tensor_copy(ctxT[:, s, :], pt[:, 0:16])
            addq16 = pOut.tile([16, SD], bf16, tag="addq", bufs=2)
            nc.sync.dma_start(addq16[:], addq_d[a * 16:(a + 1) * 16, :])
            cag = pOut.tile([16, SD], f32, tag="cagg", bufs=2)
            for q8 in range(8):
                owt = pW.tile([128, KT_SD, 256], bf16, tag="wquart")
                nc.sync.dma_start(owt[:], ow_d[a, q8])
                po = psum.tile([16, 256], f32, **QUAD)
                for k in range(KT_SD):
                    nc.tensor.matmul(po[:], ctxT[:, k, :], owt[:, k, :],
                                     start=(k == 0), stop=(k == KT_SD - 1))
                nc.vector.tensor_add(cag[:, q8 * 256:(q8 + 1) * 256],
                                     po[:], addq16[:, q8 * 256:(q8 + 1) * 256])
            # ---------- final LN (per adapter) ----------
            pg16 = pOut.tile([16, SD], bf16, tag="pg", bufs=2)
            nc.sync.dma_start(pg16[:], pg_d[a * 16:(a + 1) * 16, :])
            pb16 = pOut.tile([16, SD], bf16, tag="pb", bufs=2)
            nc.sync.dma_start(pb16[:], pb_d[a * 16:(a + 1) * 16, :])
            s1 = pSt.tile([16, 1], f32, tag="st16")
            nc.vector.tensor_reduce(s1[:], cag[:], axis=mybir.AxisListType.X, op=ALU.add)
            negmu = pSt.tile([16, 1], f32, tag="st16")
            nc.vector.tensor_scalar_mul(negmu[:], s1[:], -1.0 / SD)
            xh = pOut.tile([16, SD], bf16, tag="xh", bufs=2)
            vs = pSt.tile([16, 1], f32, tag="st16")
            nc.scalar.activation(xh[:], cag[:], AF.Square, bias=negmu[:], scale=1.0,
                                 accum_out=vs[:])
            sd_ = pSt.tile([16, 1], f32, tag="st16")
            nc.scalar.activation(sd_[:], vs[:], AF.Sqrt, bias=eps_sb[0:16],
                                 scale=1.0 / SD)
            rstd = pSt.tile([16, 1], f32, tag="st16")
            nc.vector.reciprocal(rstd[:], sd_[:])
            nmr = pSt.tile([16, 1], f32, tag="st16")
            nc.vector.scalar_tensor_tensor(nmr[:], negmu[:], 1.0, rstd[:],
                                           ALU.mult, ALU.mult)
            nc.scalar.activation(xh[:], cag[:], AF.Identity, bias=nmr[:], scale=rstd[:])
            nc.vector.tensor_mul(cag[:], xh[:], pg16[:])
            nc.vector.tensor_add(cag[:], cag[:], pb16[:])
            nc.sync.dma_start(out_d[a * 16:(a + 1) * 16, :], cag[:])

    if split:
        _split_multi_waits(nc)
    return nc


def _bf(x):
    return np.ascontiguousarray(x.astype(ml_dtypes.bfloat16))


def _f32(x):
    return np.ascontiguousarray(x.astype(np.float32))


def _prep_shared(inp):
    def W(nm, a):
        return np.asarray(inp['g_' + nm] if a == 0 else inp['s_' + nm][a - 1],
                          dtype=np.float32)

    dw = np.stack([W('dw', a).reshape(KT_TD, 128, 2, 512).transpose(2, 0, 1, 3)
                   for a in range(NA)])
    uw = np.stack([W('uw', a).reshape(KT_BD, 128, 2, 1024).transpose(2, 1, 0, 3)
                   for a in range(NA)])  # [a, h, 128, k, 1024]
    wk_l, wv_l, ow_l, wq_l = [], [], [], []
    bq_l, db_l, ub_l, addq_l, pg_l, pb_l, q_l = [], [], [], [], [], [], []
    for a in range(NA):
        lg, lb = W('lg', a), W('lb', a)
        wk_eff = lg[:, None] * W('wk', a)
        wv_eff = lg[:, None] * W('wv', a)
        ow_, ob_, bv_ = W('ow', a), W('ob', a), W('bv', a)
        q_ = W('q', a)
        # [hh, p, part(sd_in), k, col]
        wk_l.append(wk_eff.reshape(KT_SD, 128, 2, 8, 128).transpose(2, 3, 1, 0, 4))
        wv_l.append(wv_eff.reshape(KT_SD, 128, 8, 256).transpose(2, 1, 0, 3))
        ow_l.append(ow_.reshape(KT_SD, 128, 8, 256).transpose(2, 1, 0, 3))
        wq_l.append(W('wq', a).reshape(KT_SD, 128, 8, 256).transpose(2, 1, 0, 3))
        bq_l.append((W('bq', a) / 16.0).reshape(KT_SD, 128).T)
        db_l.append(W('db', a).reshape(KT_BD, 128).T)
        ub_l.append(W('ub', a)[None, :])
        # vp constant term (lb@wv + bv) flows through softmax (rows sum to 1)
        # then ow into the residual; ob adds directly.
        ob_eff = (lb @ W('wv', a) + bv_) @ ow_ + ob_
        addq_l.append(q_ + ob_eff[None, :])
        pg_l.append(np.repeat(W('pg', a)[None, :], K, 0))
        pb_l.append(np.repeat(W('pb', a)[None, :], K, 0))
        q_l.append(q_)
    q_all = np.concatenate(q_l, 0)              # [80, SD]
    qT = (q_all.T / 16.0).reshape(KT_SD, 128, 80).transpose(1, 0, 2)

    return {
        'dw': _bf(dw), 'uw': _bf(uw),
        'wk': _bf(np.stack(wk_l)), 'wv': _bf(np.stack(wv_l)),
        'ow': _bf(np.stack(ow_l)), 'wq': _bf(np.stack(wq_l)),
        'qT': _bf(qT), 'bq': _f32(np.stack(bq_l)), 'db': _f32(np.stack(db_l)),
        'ub': _bf(np.stack(ub_l)), 'addq': _bf(np.concatenate(addq_l, 0)),
        'pg': _bf(np.concatenate(pg_l, 0)), 'pb': _bf(np.concatenate(pb_l, 0)),
    }


def _gelu(x):
    from scipy.special import erf
    return (x * 0.5 * (1.0 + erf(x / np.sqrt(np.float32(2.0))))).astype(np.float32)


def kernel(**inputs):
    if 'nc' not in _CACHE:
        _CACHE['nc'] = _build()
    nc = _CACHE['nc']

    shared = _prep_shared(inputs)
    h = np.asarray(inputs['h_teacher'], dtype=np.float32)
    in_maps = []
    for b in range(B):
        hT = _bf(h[b].T.reshape(KT_TD, 128, 2, 512).transpose(2, 0, 1, 3))
        in_maps.append({**shared, 'hT': hT})

    _CACHE['in_maps'] = in_maps
    res = run_bass_kernel_spmd(nc, in_maps, core_ids=list(range(8)))
    outs = np.stack([res.results[i]['out'] for i in range(B)])  # [8, 80, 2048]

    c_g = outs[:, 0:K, :]
    c_spec = outs[:, K:, :].reshape(B, M, K, SD)

    # host router (fp32, matches reference numerics)
    pooled = h.mean(axis=1)
    r1 = _gelu(pooled @ inputs['r_w1'] + inputs['r_b1'])
    logits = (r1 @ inputs['r_w2'] + inputs['r_b2']).astype(np.float32)
    z = logits - logits.max(-1, keepdims=True)
    ez = np.exp(z)
    probs = (ez / ez.sum(-1, keepdims=True)).astype(np.float32)
    idx = np.argsort(-probs, axis=-1, kind='stable')[:, :TOPK]
    w = np.take_along_axis(probs, idx, axis=-1)
    w = w / (w.sum(-1, keepdims=True) + np.float32(1e-8))
    sel = c_spec[np.arange(B)[:, None], idx]            # [B, TOPK, K, SD]
    c_s = (sel * w[:, :, None, None]).sum(axis=1)
    c_agg = np.concatenate([c_g, c_s], axis=1).astype(np.float32)
    return c_agg, probs.astype(np.float32)


# revision 16
# speedup vs baseline: 1.4242x; 1.4242x over previous
"""AdapterBank kernel for 8 TRN2 NeuronCores.

Strategy: data-parallel over B=8 (one batch element per core, no
collectives). Each core computes all 5 adapters (1 generalized + 4
specialized) on its h_teacher slice in bf16 matmuls with fp32 PSUM
accumulation, returning [80, 2048] = 5 adapters x [K=16, SD=2048].
The tiny router MLP (0.003% of FLOPs) + top-k gather run on host in
fp32 — router logit gaps are ~4e-4, so low-precision on-device routing
would mis-select experts.

Algebraic folds (exact):
  - scores scale 1/sqrt(HD) folded into qT (host).
  - bk dropped: softmax is invariant to per-row constants.
  - bv folded into ob_eff = bv @ ow + ob (softmax rows sum to 1).
  - LN(x)*lg+lb before wk/wv folded into wk_eff = lg[:,None]*wk (and
    bk_eff absorbed as above); lb contributes lb@wk which is constant
    along rows -> dropped for wk (softmax) and folded into ob for wv.
"""

import sys
sys.path.insert(0, '/opt/trn_rl_repo')

import numpy as np
import ml_dtypes

import concourse.bass as bass
import concourse.mybir as mybir
import concourse.tile as tile
from concourse.bass_utils import run_bass_kernel_spmd
from concourse.masks import make_identity

f32 = mybir.dt.float32
bf16 = mybir.dt.bfloat16
AF = mybir.ActivationFunctionType
ALU = mybir.AluOpType

B, N, TD, SD, BD, K, M, TOPK, H = 8, 1024, 5120, 2048, 1024, 16, 4, 2, 8
HD = SD // H
NA = M + 1          # adapters: [g, s0..s3]
KT_TD = TD // 128   # 40
KT_SD = SD // 128   # 16
KT_BD = BD // 128   # 8
RT = N // 128       # 8 row tiles
EPS = 1e-5

_CACHE = {}


def _split_multi_waits(nc):
    """This container's walrus accepts only one sync-wait per instruction;
    hoist extras into standalone EventSemaphore insts on the same engine."""
    for fn in nc.m.functions:
        for bb in fn.blocks:
            out = []
            for inst in bb.instructions:
                si = inst.sync_info
                if si is not None and len(si.on_wait) > 1:
                    waits = list(si.on_wait)
                    for j, w in enumerate(waits[:-1]):
                        out.append(mybir.InstEventSemaphore(
                            name=f"{inst.name}_w{j}",
                            engine=inst.engine,
                            sync_info=mybir.SyncInfo(on_wait=[w], on_update=[]),
                        ))
                    inst.sync_info = mybir.SyncInfo(
                        on_wait=[waits[-1]], on_update=list(si.on_update))
                out.append(inst)
            bb.instructions[:] = out


def _build(split=True):
    nc = bass.Bass("TRN2", target_bir_lowering=False, debug=False, num_devices=8)

    dm = lambda nm, shp, dt: nc.dram_tensor(nm, shp, dt, kind="ExternalInput").ap()
    hT_d = dm("hT", [2, KT_TD, 128, 512], bf16)
    dw_d = dm("dw", [NA, KT_TD, 128, 1024], bf16)
    uw_d = dm("uw", [NA, 2, 128, KT_BD, 1024], bf16)
    wk_d = dm("wk", [NA, 2, 8, 128, KT_SD, 128], bf16)
    wv_d = dm("wv", [NA, 8, 128, KT_SD, 256], bf16)
    ow_d = dm("ow", [NA, 8, 128, KT_SD, 256], bf16)
    wq_d = dm("wq", [NA, 8, 128, KT_SD, 256], bf16)
    qT_d = dm("qT", [128, KT_SD, 80], bf16)
    bq_d = dm("bq", [NA, 128, KT_SD], f32)
    db_d = dm("db", [NA, 128, KT_BD], f32)
    ub_d = dm("ub", [NA, 1, SD], bf16)
    addq_d = dm("addq", [80, SD], bf16)
    pg_d = dm("pg", [80, SD], bf16)
    pb_d = dm("pb", [80, SD], bf16)
    out_d = nc.dram_tensor("out", [80, SD], f32, kind="ExternalOutput").ap()

    from contextlib import ExitStack
    with tile.TileContext(nc) as tc, ExitStack() as es:
        P_ = lambda **kw: es.enter_context(tc.tile_pool(**kw))
        # ---- pools (KB/partition noted) ----
        pW = P_(name="pW", bufs=3)     # w quarters [128,16,512]bf16 16 -> 32
        pWk = P_(name="pWk", bufs=2)   # wk cols [128,16,128]bf16 4 / hT [128,512] -> 8
        pDw = P_(name="pDw", bufs=2)  # [128,1024] 2KB   # dw rows [128,1024]bf16 2 -> 6
        pX1 = P_(name="pX1", bufs=1)   # x1T [128,8,1024]bf16 16
        pUw = P_(name="pUw", bufs=2)   # uw half [128,8,1024]bf16 16 -> 32
        pUb = P_(name="pUb", bufs=1)   # ub [128,2048]f32 8
        pX2 = P_(name="pX2", bufs=2)   # x2pre/x2hat [128,2048]bf16 4 -> 12
        pX2T = P_(name="pX2T", bufs=1) # x2T [128,16,1024]bf16 32
        pKV = P_(name="pKV", bufs=1)   # kpT/vp half [128,8,1024]bf16 16 -> 32
        pQ = P_(name="pQ", bufs=1)     # qT resident 2.5 + qp 4 + qpT 0.5
        pAtt = P_(name="pAtt", bufs=1) # att tiles small
        pCtx = P_(name="pCtx", bufs=1) # ctx [16,2048]bf16 4 + ctxT 0.5
        pOut = P_(name="pOut", bufs=1) # cagg 8 + addq 8 + pg/pb 8 + xh 4
        pSt = P_(name="pSt", bufs=8)  # [128,1] stats
        pB = P_(name="pB", bufs=1)     # db/bq consts

        psum = P_(name="psum", bufs=1, space="PSUM")
        # tag "quad": [128,512]-max tiles, 4 slots x 1 bank
        # tag "wide2": [128,1024]-max tiles, 2 slots x 2 banks
        QUAD = dict(tag="quad", bufs=4)
        WIDE2 = dict(tag="wide2", bufs=2)

        # ---- persistent tiles ----
        ident = pB.tile([128, 128], bf16, tag="ident")
        make_identity(nc, ident[:])
        eps_sb = pB.tile([128, 1], f32, tag="eps")
        nc.vector.memset(eps_sb[:], EPS)
        qT_sb = pQ.tile([128, KT_SD, 80], bf16, tag="qT")
        nc.sync.dma_start(qT_sb[:], qT_d[:])

        for a in range(NA):
            # ---------- A: x1T = gelu(h @ dw + db).T  -> [BD, N] ----------
            db_sb = pB.tile([128, KT_BD], f32, tag="db", bufs=2)
            nc.sync.dma_start(db_sb[:], db_d[a])
            x1T = pX1.tile([128, KT_BD, N], bf16, tag="x1T")
            for rh in range(2):
                psa = [psum.tile([128, 512], f32, name=f"psa_{a}_{rh}_{p}", **QUAD)
                       for p in range(4)]
                paw = [psum.tile([128, 1024], f32, name=f"paw_{a}_{rh}_{w}", **WIDE2)
                       for w in range(2)]
                for k in range(KT_TD):
                    ht = pWk.tile([128, 512], bf16, tag="wkcol")
                    nc.sync.dma_start(ht[:], hT_d[rh, k])
                    dwt = pDw.tile([128, 1024], bf16, tag="dw")
                    nc.sync.dma_start(dwt[:], dw_d[a, k])
                    for p in range(8):
                        dst = (psa[p][:] if p < 4 else
                               paw[(p - 4) // 2][:, ((p - 4) % 2) * 512:((p - 4) % 2 + 1) * 512])
                        nc.tensor.matmul(dst, dwt[:, p * 128:(p + 1) * 128],
                                         ht[:], start=(k == 0), stop=(k == KT_TD - 1))
                for p in range(8):
                    srcp = (psa[p][:] if p < 4 else
                            paw[(p - 4) // 2][:, ((p - 4) % 2) * 512:((p - 4) % 2 + 1) * 512])
                    nc.scalar.activation(x1T[:, p, rh * 512:(rh + 1) * 512],
                                         srcp, AF.Gelu,
                                         bias=db_sb[:, p:p + 1], scale=1.0)

            # ---------- B+C: x2 = LN(x1 @ uw + ub); x2T ----------
            ub_sb = pUb.tile([128, SD], bf16, tag="ub")
            nc.sync.dma_start(ub_sb[:], ub_d[a].broadcast_to((128, SD)))
            uw_sb = []
            for hf in range(2):
                t = pUw.tile([128, KT_BD, 1024], bf16, tag="uw")
                nc.sync.dma_start(t[:], uw_d[a, hf])
                uw_sb.append(t)
            x2T = pX2T.tile([128, KT_SD, N], bf16, tag="x2T")
            for r in range(RT):
                x2pre = pX2.tile([128, SD], bf16, tag="x2")
                s1h = []
                for hf in range(2):
                    ps2 = psum.tile([128, 1024], f32, name=f"ps2_{a}_{r}_{hf}", **WIDE2)
                    for k in range(KT_BD):
                        lhs = x1T[:, k, r * 128:(r + 1) * 128]
                        for ch in range(2):
                            nc.tensor.matmul(
                                ps2[:, ch * 512:(ch + 1) * 512],
                                lhs, uw_sb[hf][:, k, ch * 512:(ch + 1) * 512],
                                start=(k == 0), stop=(k == KT_BD - 1))
                    sh = pSt.tile([128, 1], f32, tag="st")
                    nc.vector.scalar_tensor_tensor(
                        x2pre[:, hf * 1024:(hf + 1) * 1024], ps2[:], 1.0,
                        ub_sb[:, hf * 1024:(hf + 1) * 1024],
                        ALU.mult, ALU.add, accum_out=sh[:])
                    s1h.append(sh)
                s1 = pSt.tile([128, 1], f32, tag="st")
                nc.vector.tensor_add(s1[:], s1h[0][:], s1h[1][:])
                negmu = pSt.tile([128, 1], f32, tag="st")
                nc.vector.tensor_scalar_mul(negmu[:], s1[:], -1.0 / SD)
                # var: Square(x2pre - mu) -> accum vs ; throwaway full out
                x2hat = pX2.tile([128, SD], bf16, tag="x2")
                vs = pSt.tile([128, 1], f32, tag="st")
                nc.scalar.activation(x2hat[:], x2pre[:], AF.Square,
                                     bias=negmu[:], scale=1.0, accum_out=vs[:])
                sd_ = pSt.tile([128, 1], f32, tag="st")
                nc.scalar.activation(sd_[:], vs[:], AF.Sqrt, bias=eps_sb[:],
                                     scale=1.0 / SD)
                rstd = pSt.tile([128, 1], f32, tag="st")
                nc.vector.reciprocal(rstd[:], sd_[:])
                nmr = pSt.tile([128, 1], f32, tag="st")
                nc.vector.scalar_tensor_tensor(nmr[:], negmu[:], 1.0, rstd[:],
                                               ALU.mult, ALU.mult)
                nc.scalar.activation(x2hat[:], x2pre[:], AF.Identity,
                                     bias=nmr[:], scale=rstd[:])
                # C: transpose into x2T
                for s in range(KT_SD):
                    pt = psum.tile([128, 128], bf16, **QUAD)
                    nc.tensor.transpose(pt[:], x2hat[:, s * 128:(s + 1) * 128], ident[:])
                    nc.vector.tensor_copy(x2T[:, s, r * 128:(r + 1) * 128], pt[:])

            # ---------- F: qp = (q @ wq + bq)/16 ; qpT ----------
            bq_sb = pB.tile([128, KT_SD], f32, tag="bq", bufs=2)
            nc.sync.dma_start(bq_sb[:], bq_d[a])
            qp_sb = pQ.tile([16, SD], bf16, tag="qp")
            for q8 in range(8):
                wqt = pW.tile([128, KT_SD, 256], bf16, tag="wquart")
                nc.sync.dma_start(wqt[:], wq_d[a, q8])
                psq = psum.tile([16, 256], f32, name=f"psq_{a}_{q8}", **QUAD)
                for k in range(KT_SD):
                    nc.tensor.matmul(psq[:],
                                     qT_sb[:, k, a * 16:(a + 1) * 16], wqt[:, k, :],
                                     start=(k == 0), stop=(k == KT_SD - 1))
                nc.vector.tensor_copy(qp_sb[:, q8 * 256:(q8 + 1) * 256], psq[:])
            qpT = pQ.tile([128, KT_SD, 16], bf16, tag="qpT")
            for s in range(KT_SD):
                pt = psum.tile([128, 128], bf16, **QUAD)
                nc.tensor.transpose(pt[:, 0:16], qp_sb[:, s * 128:(s + 1) * 128], ident[0:16, 0:16])
                nc.scalar.activation(qpT[:, s, :], pt[:, 0:16], AF.Identity,
                                     bias=bq_sb[:, s:s + 1], scale=1.0)

            ctx_sb = pCtx.tile([16, SD], bf16, tag="ctx")
            for hh in range(2):
                # ---------- D: kpT half ----------
                kpT = pKV.tile([128, 8, N], bf16, tag="kv")
                for p in range(8):
                    wkt = pWk.tile([128, KT_SD, 128], bf16, tag="wkcol")
                    nc.sync.dma_start(wkt[:], wk_d[a, hh, p])
                    pkv = psum.tile([128, 1024], f32, name=f"pkv_{a}_{hh}_{p}", **WIDE2)
                    for k in range(KT_SD):
                        for ch in range(2):
                            nc.tensor.matmul(pkv[:, ch * 512:(ch + 1) * 512],
                                             wkt[:, k, :], x2T[:, k, ch * 512:(ch + 1) * 512],
                                             start=(k == 0), stop=(k == KT_SD - 1))
                    nc.vector.tensor_copy(kpT[:, p, :], pkv[:])
                # ---------- G (scores): head hl at rows hl*32..hl*32+16
                # (32-aligned starts); unused rows zeroed -> exp(0)=1, harmless.
                att = pAtt.tile([128, N], bf16, tag="att")
                nc.vector.memset(att[:], 0.0)
                for hl in range(4):
                    pss = [psum.tile([16, 512], f32, name=f"pss_{a}_{hh}_{hl}_{c}", **QUAD)
                           for c in range(2)]
                    for j in range(2):
                        st_g = (hh * 4 + hl) * 2 + j
                        for ch in range(2):
                            nc.tensor.matmul(pss[ch][:],
                                             qpT[:, st_g, :],
                                             kpT[:, hl * 2 + j, ch * 512:(ch + 1) * 512],
                                             start=(j == 0), stop=(j == 1))
                    for ch in range(2):
                        nc.vector.tensor_copy(
                            att[hl * 32:hl * 32 + 16, ch * 512:(ch + 1) * 512], pss[ch][:])
                # ---------- E: vp half (PE works while softmax runs on ACT/DVE)
                vp = pKV.tile([128, RT, 1024], bf16, tag="kv")
                for q8 in range(hh * 4, hh * 4 + 4):
                    wvt = pW.tile([128, KT_SD, 256], bf16, tag="wquart")
                    nc.sync.dma_start(wvt[:], wv_d[a, q8])
                    cc = (q8 % 4) * 256
                    for r in range(RT):
                        pv = psum.tile([128, 256], f32, **QUAD)
                        for k in range(KT_SD):
                            nc.tensor.matmul(pv[:], x2T[:, k, r * 128:(r + 1) * 128],
                                             wvt[:, k, :],
                                             start=(k == 0), stop=(k == KT_SD - 1))
                        nc.vector.tensor_copy(vp[:, r, cc:cc + 256], pv[:])
                # ---------- softmax + attT (overlaps E on PE) ----------
                # scores are O(1): exp without max-subtract is safe in f32
                atte = pAtt.tile([128, N], bf16, tag="atte")
                se = pSt.tile([128, 1], f32, tag="st")
                nc.scalar.activation(atte[:], att[:], AF.Exp, accum_out=se[:])
                rs = pSt.tile([128, 1], f32, tag="st")
                nc.vector.reciprocal(rs[:], se[:])
                attn = pAtt.tile([128, N], bf16, tag="attn")
                nc.vector.tensor_scalar_mul(attn[:], atte[:], rs[:])
                attT = pAtt.tile([128, RT, 128], bf16, tag="attT")
                for r in range(RT):
                    pt = psum.tile([128, 128], bf16, name=f"ptt_{a}_{hh}_{r}", **QUAD)
                    nc.tensor.transpose(pt[:], attn[:, r * 128:(r + 1) * 128],
                                        ident[:])
                    nc.vector.tensor_copy(attT[:, r, :], pt[:])
                # ---------- H: ctx half ----------
                pc = [psum.tile([16, 256], f32, name=f"pc_{a}_{hh}_{c}", **QUAD)
                      for c in range(4)]
                for r in range(RT):
                    for hl in range(4):
                        nc.tensor.matmul(pc[hl][:],
                                         attT[:, r, hl * 32:hl * 32 + 16],
                                         vp[:, r, hl * 256:(hl + 1) * 256],
                                         start=(r == 0), stop=(r == RT - 1))
                for c in range(4):
                    nc.vector.tensor_copy(
                        ctx_sb[:, hh * 1024 + c * 256:hh * 1024 + (c + 1) * 256], pc[c][:])

            # ---------- ctxT + I: ctx2 = ctx @ ow (+addq) ----------
            ctxT = pCtx.tile([128, KT_SD, 16], bf16, tag="ctxT")
            for s in range(KT_SD):
                pt = psum.tile([128, 128], bf16, **QUAD)
                nc.tensor.transpose(pt[:, 0:16], ctx_sb[:, s * 128:(s + 1) * 128],
                                    ident[0:16, 0:16])
                nc.vector.tensor_copy(ctxT[:, s, :], pt[:, 0:16])
            addq16 = pOut.tile([16, SD], bf16, tag="addq", bufs=1)
            nc.sync.dma_start(addq16[:], addq_d[a * 16:(a + 1) * 16, :])
            cag = pOut.tile([16, SD], f32, tag="cagg", bufs=2)
            for q8 in range(8):
                owt = pW.tile([128, KT_SD, 256], bf16, tag="wquart")
                nc.sync.dma_start(owt[:], ow_d[a, q8])
                po = psum.tile([16, 256], f32, **QUAD)
                for k in range(KT_SD):
                    nc.tensor.matmul(po[:], ctxT[:, k, :], owt[:, k, :],
                                     start=(k == 0), stop=(k == KT_SD - 1))
                nc.vector.tensor_add(cag[:, q8 * 256:(q8 + 1) * 256],
                                     po[:], addq16[:, q8 * 256:(q8 + 1) * 256])
            # ---------- final LN (per adapter) ----------
            pg16 = pOut.tile([16, SD], bf16, tag="pg", bufs=1)
            nc.sync.dma_start(pg16[:], pg_d[a * 16:(a + 1) * 16, :])
            pb16 = pOut.tile([16, SD], bf16, tag="pb", bufs=1)
            nc.sync.dma_start(pb16[:], pb_d[a * 16:(a + 1) * 16, :])
            s1 = pSt.tile([16, 1], f32, tag="st16")
            nc.vector.tensor_reduce(s1[:], cag[:], axis=mybir.AxisListType.X, op=ALU.add)
            negmu = pSt.tile([16, 1], f32, tag="st16")
            nc.vector.tensor_scalar_mul(negmu[:], s1[:], -1.0 / SD)
            xh = pOut.tile([16, SD], bf16, tag="xh", bufs=1)
            vs = pSt.tile([16, 1], f32, tag="st16")
            nc.scalar.activation(xh[:], cag[:], AF.Square, bias=negmu[:], scale=1.0,
                                 accum_out=vs[:])
            sd_ = pSt.tile([16, 1], f32, tag="st16")
            nc.scalar.activation(sd_[:], vs[:], AF.Sqrt, bias=eps_sb[0:16],
                                 scale=1.0 / SD)
            rstd = pSt.tile([16, 1], f32, tag="st16")
            nc.vector.reciprocal(rstd[:], sd_[:])
            nmr = pSt.tile([16, 1], f32, tag="st16")
            nc.vector.scalar_tensor_tensor(nmr[:], negmu[:], 1.0, rstd[:],
                                           ALU.mult, ALU.mult)
            nc.scalar.activation(xh[:], cag[:], AF.Identity, bias=nmr[:], scale=rstd[:])
            nc.vector.tensor_mul(cag[:], xh[:], pg16[:])
            nc.vector.tensor_add(cag[:], cag[:], pb16[:])
            nc.sync.dma_start(out_d[a * 16:(a + 1) * 16, :], cag[:])

    if split:
        _split_multi_waits(nc)
    return nc


def _bf(x):
    return np.ascontiguousarray(x.astype(ml_dtypes.bfloat16))


def _f32(x):
    return np.ascontiguousarray(x.astype(np.float32))


def _prep_shared(inp):
    def W(nm, a):
        return np.asarray(inp['g_' + nm] if a == 0 else inp['s_' + nm][a - 1],
                          dtype=np.float32)

    dw = np.stack([W('dw', a).reshape(KT_TD, 128, 1024) for a in range(NA)])
    uw = np.stack([W('uw', a).reshape(KT_BD, 128, 2, 1024).transpose(2, 1, 0, 3)
                   for a in range(NA)])  # [a, h, 128, k, 1024]
    wk_l, wv_l, ow_l, wq_l = [], [], [], []
    bq_l, db_l, ub_l, addq_l, pg_l, pb_l, q_l = [], [], [], [], [], [], []
    for a in range(NA):
        lg, lb = W('lg', a), W('lb', a)
        wk_eff = lg[:, None] * W('wk', a)
        wv_eff = lg[:, None] * W('wv', a)
        ow_, ob_, bv_ = W('ow', a), W('ob', a), W('bv', a)
        q_ = W('q', a)
        # [hh, p, part(sd_in), k, col]
        wk_l.append(wk_eff.reshape(KT_SD, 128, 2, 8, 128).transpose(2, 3, 1, 0, 4))
        wv_l.append(wv_eff.reshape(KT_SD, 128, 8, 256).transpose(2, 1, 0, 3))
        ow_l.append(ow_.reshape(KT_SD, 128, 8, 256).transpose(2, 1, 0, 3))
        wq_l.append(W('wq', a).reshape(KT_SD, 128, 8, 256).transpose(2, 1, 0, 3))
        bq_l.append((W('bq', a) / 16.0).reshape(KT_SD, 128).T)
        db_l.append(W('db', a).reshape(KT_BD, 128).T)
        ub_l.append(W('ub', a)[None, :])
        # vp constant term (lb@wv + bv) flows through softmax (rows sum to 1)
        # then ow into the residual; ob adds directly.
        ob_eff = (lb @ W('wv', a) + bv_) @ ow_ + ob_
        addq_l.append(q_ + ob_eff[None, :])
        pg_l.append(np.repeat(W('pg', a)[None, :], K, 0))
        pb_l.append(np.repeat(W('pb', a)[None, :], K, 0))
        q_l.append(q_)
    q_all = np.concatenate(q_l, 0)              # [80, SD]
    qT = (q_all.T / 16.0).reshape(KT_SD, 128, 80).transpose(1, 0, 2)

    return {
        'dw': _bf(dw), 'uw': _bf(uw),
        'wk': _bf(np.stack(wk_l)), 'wv': _bf(np.stack(wv_l)),
        'ow': _bf(np.stack(ow_l)), 'wq': _bf(np.stack(wq_l)),
        'qT': _bf(qT), 'bq': _f32(np.stack(bq_l)), 'db': _f32(np.stack(db_l)),
        'ub': _bf(np.stack(ub_l)), 'addq': _bf(np.concatenate(addq_l, 0)),
        'pg': _bf(np.concatenate(pg_l, 0)), 'pb': _bf(np.concatenate(pb_l, 0)),
    }


def _gelu(x):
    from scipy.special import erf
    return (x * 0.5 * (1.0 + erf(x / np.sqrt(np.float32(2.0))))).astype(np.float32)


def kernel(**inputs):
    if 'nc' not in _CACHE:
        _CACHE['nc'] = _build()
    nc = _CACHE['nc']

    shared = _prep_shared(inputs)
    h = np.asarray(inputs['h_teacher'], dtype=np.float32)
    in_maps = []
    for b in range(B):
        hT = _bf(h[b].T.reshape(KT_TD, 128, 2, 512).transpose(2, 0, 1, 3))
        in_maps.append({**shared, 'hT': hT})

    _CACHE['in_maps'] = in_maps
    res = run_bass_kernel_spmd(nc, in_maps, core_ids=list(range(8)))
    outs = np.stack([res.results[i]['out'] for i in range(B)])  # [8, 80, 2048]

    c_g = outs[:, 0:K, :]
    c_spec = outs[:, K:, :].reshape(B, M, K, SD)

    # host router (fp32, matches reference numerics)
    pooled = h.mean(axis=1)
    r1 = _gelu(pooled @ inputs['r_w1'] + inputs['r_b1'])
    logits = (r1 @ inputs['r_w2'] + inputs['r_b2']).astype(np.float32)
    z = logits - logits.max(-1, keepdims=True)
    ez = np.exp(z)
    probs = (ez / ez.sum(-1, keepdims=True)).astype(np.float32)
    idx = np.argsort(-probs, axis=-1, kind='stable')[:, :TOPK]
    w = np.take_along_axis(probs, idx, axis=-1)
    w = w / (w.sum(-1, keepdims=True) + np.float32(1e-8))
    sel = c_spec[np.arange(B)[:, None], idx]            # [B, TOPK, K, SD]
    c_s = (sel * w[:, :, None, None]).sum(axis=1)
    c_agg = np.concatenate([c_g, c_s], axis=1).astype(np.float32)
    return c_agg, probs.astype(np.float32)


# revision 18
# speedup vs baseline: 1.5819x; 1.1107x over previous
"""AdapterBank kernel for 8 TRN2 NeuronCores.

Strategy: data-parallel over B=8 (one batch element per core, no
collectives). Each core computes all 5 adapters (1 generalized + 4
specialized) on its h_teacher slice in bf16 matmuls with fp32 PSUM
accumulation, returning [80, 2048] = 5 adapters x [K=16, SD=2048].
The tiny router MLP (0.003% of FLOPs) + top-k gather run on host in
fp32 — router logit gaps are ~4e-4, so low-precision on-device routing
would mis-select experts.

Algebraic folds (exact):
  - scores scale 1/sqrt(HD) folded into qT (host).
  - bk dropped: softmax is invariant to per-row constants.
  - bv folded into ob_eff = bv @ ow + ob (softmax rows sum to 1).
  - LN(x)*lg+lb before wk/wv folded into wk_eff = lg[:,None]*wk (and
    bk_eff absorbed as above); lb contributes lb@wk which is constant
    along rows -> dropped for wk (softmax) and folded into ob for wv.
"""

import sys
sys.path.insert(0, '/opt/trn_rl_repo')

import numpy as np
import ml_dtypes

import concourse.bass as bass
import concourse.mybir as mybir
import concourse.tile as tile
from concourse.bass_utils import run_bass_kernel_spmd
from concourse.masks import make_identity

f32 = mybir.dt.float32
bf16 = mybir.dt.bfloat16
AF = mybir.ActivationFunctionType
ALU = mybir.AluOpType

B, N, TD, SD, BD, K, M, TOPK, H = 8, 1024, 5120, 2048, 1024, 16, 4, 2, 8
HD = SD // H
NA = M + 1          # adapters: [g, s0..s3]
KT_TD = TD // 128   # 40
KT_SD = SD // 128   # 16
KT_BD = BD // 128   # 8
RT = N // 128       # 8 row tiles
EPS = 1e-5

_CACHE = {}


def _split_multi_waits(nc):
    """This container's walrus accepts only one sync-wait per instruction;
    hoist extras into standalone EventSemaphore insts on the same engine."""
    for fn in nc.m.functions:
        for bb in fn.blocks:
            out = []
            for inst in bb.instructions:
                si = inst.sync_info
                if si is not None and len(si.on_wait) > 1:
                    waits = list(si.on_wait)
                    for j, w in enumerate(waits[:-1]):
                        out.append(mybir.InstEventSemaphore(
                            name=f"{inst.name}_w{j}",
                            engine=inst.engine,
                            sync_info=mybir.SyncInfo(on_wait=[w], on_update=[]),
                        ))
                    inst.sync_info = mybir.SyncInfo(
                        on_wait=[waits[-1]], on_update=list(si.on_update))
                out.append(inst)
            bb.instructions[:] = out


def _build(split=True):
    nc = bass.Bass("TRN2", target_bir_lowering=False, debug=False, num_devices=8)

    dm = lambda nm, shp, dt: nc.dram_tensor(nm, shp, dt, kind="ExternalInput").ap()
    hT_d = dm("hT", [2, KT_TD, 128, 512], bf16)
    dw_d = dm("dw", [NA, KT_TD, 128, 1024], bf16)
    uw_d = dm("uw", [NA, 2, 128, KT_BD, 1024], bf16)
    wk_d = dm("wk", [NA, 2, 8, 128, KT_SD, 128], bf16)
    wv_d = dm("wv", [NA, 8, 128, KT_SD, 256], bf16)
    ow_d = dm("ow", [NA, 8, 128, KT_SD, 256], bf16)
    wq_d = dm("wq", [NA, 8, 128, KT_SD, 256], bf16)
    qT_d = dm("qT", [128, KT_SD, 80], bf16)
    bq_d = dm("bq", [NA, 128, KT_SD], f32)
    db_d = dm("db", [NA, 128, KT_BD], f32)
    ub_d = dm("ub", [NA, 1, SD], bf16)
    addq_d = dm("addq", [80, SD], bf16)
    pg_d = dm("pg", [80, SD], bf16)
    pb_d = dm("pb", [80, SD], bf16)
    out_d = nc.dram_tensor("out", [80, SD], f32, kind="ExternalOutput").ap()

    from contextlib import ExitStack
    with tile.TileContext(nc) as tc, ExitStack() as es:
        P_ = lambda **kw: es.enter_context(tc.tile_pool(**kw))
        # ---- pools (KB/partition noted) ----
        pW = P_(name="pW", bufs=3)     # w quarters [128,16,512]bf16 16 -> 32
        pWk = P_(name="pWk", bufs=2)   # wk cols [128,16,128]bf16 4 / hT [128,512] -> 8
        pDw = P_(name="pDw", bufs=2)  # [128,2,1024] 4KB   # dw rows [128,1024]bf16 2 -> 6
        pX1 = P_(name="pX1", bufs=1)   # x1T [128,8,1024]bf16 16
        pUw = P_(name="pUw", bufs=2)   # uw half [128,8,1024]bf16 16 -> 32
        pUb = P_(name="pUb", bufs=1)   # ub [128,2048]f32 8
        pX2 = P_(name="pX2", bufs=3)   # x2pre/x2hat [128,2048]bf16 4 -> 12
        pX2T = P_(name="pX2T", bufs=1) # x2T [128,16,1024]bf16 32
        pKV = P_(name="pKV", bufs=1)   # kpT/vp half [128,8,1024]bf16 16 -> 32
        pQ = P_(name="pQ", bufs=1)     # qT resident 2.5 + qp 4 + qpT 0.5
        pAtt = P_(name="pAtt", bufs=1) # att tiles small
        pCtx = P_(name="pCtx", bufs=1) # ctx [16,2048]bf16 4 + ctxT 0.5
        pOut = P_(name="pOut", bufs=1) # cagg 8 + addq 8 + pg/pb 8 + xh 4
        pSt = P_(name="pSt", bufs=8)  # [128,1] stats
        pB = P_(name="pB", bufs=1)     # db/bq consts

        psum = P_(name="psum", bufs=1, space="PSUM")
        # tag "quad": [128,512]-max tiles, 4 slots x 1 bank
        # tag "wide2": [128,1024]-max tiles, 2 slots x 2 banks
        QUAD = dict(tag="quad", bufs=4)
        WIDE2 = dict(tag="wide2", bufs=2)

        # ---- persistent tiles ----
        ident = pB.tile([128, 128], bf16, tag="ident")
        make_identity(nc, ident[:])
        eps_sb = pB.tile([128, 1], f32, tag="eps")
        nc.vector.memset(eps_sb[:], EPS)
        qT_sb = pQ.tile([128, KT_SD, 80], bf16, tag="qT")
        nc.sync.dma_start(qT_sb[:], qT_d[:])

        for a in range(NA):
            # ---------- A: x1T = gelu(h @ dw + db).T  -> [BD, N] ----------
            db_sb = pB.tile([128, KT_BD], f32, tag="db", bufs=2)
            nc.sync.dma_start(db_sb[:], db_d[a])
            x1T = pX1.tile([128, KT_BD, N], bf16, tag="x1T")
            for rh in range(2):
                psa = [psum.tile([128, 512], f32, name=f"psa_{a}_{rh}_{p}", **QUAD)
                       for p in range(4)]
                paw = [psum.tile([128, 1024], f32, name=f"paw_{a}_{rh}_{w}", **WIDE2)
                       for w in range(2)]
                for k2 in range(0, KT_TD, 2):
                    ht = pWk.tile([128, 2, 512], bf16, tag="wkcol")
                    nc.sync.dma_start(ht[:], hT_d[rh, k2:k2 + 2].rearrange('k p c -> p k c'))
                    dwt = pDw.tile([128, 2, 1024], bf16, tag="dw")
                    nc.gpsimd.dma_start(dwt[:], dw_d[a, k2:k2 + 2].rearrange('k p c -> p k c'))
                    for dk in range(2):
                        k = k2 + dk
                        for p in range(8):
                            dst = (psa[p][:] if p < 4 else
                                   paw[(p - 4) // 2][:, ((p - 4) % 2) * 512:((p - 4) % 2 + 1) * 512])
                            nc.tensor.matmul(dst, dwt[:, dk, p * 128:(p + 1) * 128],
                                             ht[:, dk, :], start=(k == 0),
                                             stop=(k == KT_TD - 1))
                for p in range(8):
                    srcp = (psa[p][:] if p < 4 else
                            paw[(p - 4) // 2][:, ((p - 4) % 2) * 512:((p - 4) % 2 + 1) * 512])
                    nc.scalar.activation(x1T[:, p, rh * 512:(rh + 1) * 512],
                                         srcp, AF.Gelu,
                                         bias=db_sb[:, p:p + 1], scale=1.0)

            # ---------- B+C: x2 = LN(x1 @ uw + ub); x2T ----------
            ub_sb = pUb.tile([128, SD], bf16, tag="ub")
            nc.sync.dma_start(ub_sb[:], ub_d[a].broadcast_to((128, SD)))
            uw_sb = []
            for hf in range(2):
                t = pUw.tile([128, KT_BD, 1024], bf16, tag="uw")
                nc.sync.dma_start(t[:], uw_d[a, hf])
                uw_sb.append(t)
            x2T = pX2T.tile([128, KT_SD, N], bf16, tag="x2T")
            pend = []  # (x2hat, r) awaiting transpose — emitted one iter later
            def flush_c():
                x2h_, r_ = pend.pop(0)
                for s in range(KT_SD):
                    pt = psum.tile([128, 128], bf16, name=f"ptc_{a}_{r_}_{s}", **QUAD)
                    nc.tensor.transpose(pt[:], x2h_[:, s * 128:(s + 1) * 128], ident[:])
                    nc.vector.tensor_copy(x2T[:, s, r_ * 128:(r_ + 1) * 128], pt[:])
            for r in range(RT):
                x2pre = pX2.tile([128, SD], bf16, tag="x2")
                s1h = []
                for hf in range(2):
                    ps2 = psum.tile([128, 1024], f32, name=f"ps2_{a}_{r}_{hf}", **WIDE2)
                    for k in range(KT_BD):
                        lhs = x1T[:, k, r * 128:(r + 1) * 128]
                        for ch in range(2):
                            nc.tensor.matmul(
                                ps2[:, ch * 512:(ch + 1) * 512],
                                lhs, uw_sb[hf][:, k, ch * 512:(ch + 1) * 512],
                                start=(k == 0), stop=(k == KT_BD - 1))
                    sh = pSt.tile([128, 1], f32, tag="st")
                    nc.vector.scalar_tensor_tensor(
                        x2pre[:, hf * 1024:(hf + 1) * 1024], ps2[:], 1.0,
                        ub_sb[:, hf * 1024:(hf + 1) * 1024],
                        ALU.mult, ALU.add, accum_out=sh[:])
                    s1h.append(sh)
                if pend:
                    flush_c()
                s1 = pSt.tile([128, 1], f32, tag="st")
                nc.vector.tensor_add(s1[:], s1h[0][:], s1h[1][:])
                negmu = pSt.tile([128, 1], f32, tag="st")
                nc.vector.tensor_scalar_mul(negmu[:], s1[:], -1.0 / SD)
                # var: Square(x2pre - mu) -> accum vs ; throwaway full out
                x2hat = pX2.tile([128, SD], bf16, tag="x2")
                vs = pSt.tile([128, 1], f32, tag="st")
                nc.scalar.activation(x2hat[:], x2pre[:], AF.Square,
                                     bias=negmu[:], scale=1.0, accum_out=vs[:])
                sd_ = pSt.tile([128, 1], f32, tag="st")
                nc.scalar.activation(sd_[:], vs[:], AF.Sqrt, bias=eps_sb[:],
                                     scale=1.0 / SD)
                rstd = pSt.tile([128, 1], f32, tag="st")
                nc.vector.reciprocal(rstd[:], sd_[:])
                nmr = pSt.tile([128, 1], f32, tag="st")
                nc.vector.scalar_tensor_tensor(nmr[:], negmu[:], 1.0, rstd[:],
                                               ALU.mult, ALU.mult)
                nc.scalar.activation(x2hat[:], x2pre[:], AF.Identity,
                                     bias=nmr[:], scale=rstd[:])
                pend.append((x2hat, r))
            while pend:
                flush_c()

            # ---------- F: qp = (q @ wq + bq)/16 ; qpT ----------
            bq_sb = pB.tile([128, KT_SD], f32, tag="bq", bufs=2)
            nc.sync.dma_start(bq_sb[:], bq_d[a])
            qp_sb = pQ.tile([16, SD], bf16, tag="qp")
            for q8 in range(8):
                wqt = pW.tile([128, KT_SD, 256], bf16, tag="wquart")
                nc.gpsimd.dma_start(wqt[:], wq_d[a, q8])
                psq = psum.tile([16, 256], f32, name=f"psq_{a}_{q8}", **QUAD)
                for k in range(KT_SD):
                    nc.tensor.matmul(psq[:],
                                     qT_sb[:, k, a * 16:(a + 1) * 16], wqt[:, k, :],
                                     start=(k == 0), stop=(k == KT_SD - 1))
                nc.vector.tensor_copy(qp_sb[:, q8 * 256:(q8 + 1) * 256], psq[:])
            qpT = pQ.tile([128, KT_SD, 16], bf16, tag="qpT")

            def emit_qpT():
                for s in range(KT_SD):
                    pt = psum.tile([128, 128], bf16, name=f"ptq_{a}_{s}", **QUAD)
                    nc.tensor.transpose(pt[:, 0:16], qp_sb[:, s * 128:(s + 1) * 128],
                                        ident[0:16, 0:16])
                    nc.scalar.activation(qpT[:, s, :], pt[:, 0:16], AF.Identity,
                                         bias=bq_sb[:, s:s + 1], scale=1.0)

            ctx_sb = pCtx.tile([16, SD], bf16, tag="ctx")
            for hh in range(2):
                # ---------- D: kpT half ----------
                kpT = pKV.tile([128, 8, N], bf16, tag="kv")
                for p in range(8):
                    wkt = pWk.tile([128, KT_SD, 128], bf16, tag="wkcol")
                    nc.sync.dma_start(wkt[:], wk_d[a, hh, p])
                    pkv = psum.tile([128, 1024], f32, name=f"pkv_{a}_{hh}_{p}", **WIDE2)
                    for k in range(KT_SD):
                        for ch in range(2):
                            nc.tensor.matmul(pkv[:, ch * 512:(ch + 1) * 512],
                                             wkt[:, k, :], x2T[:, k, ch * 512:(ch + 1) * 512],
                                             start=(k == 0), stop=(k == KT_SD - 1))
                    nc.vector.tensor_copy(kpT[:, p, :], pkv[:])
                if hh == 0:
                    emit_qpT()
                # ---------- G (scores): head hl at rows hl*32..hl*32+16
                # (32-aligned starts); unused rows zeroed -> exp(0)=1, harmless.
                att = pAtt.tile([128, N], bf16, tag="att")
                nc.vector.memset(att[:], 0.0)
                for hl in range(4):
                    pss = [psum.tile([16, 512], f32, name=f"pss_{a}_{hh}_{hl}_{c}", **QUAD)
                           for c in range(2)]
                    for j in range(2):
                        st_g = (hh * 4 + hl) * 2 + j
                        for ch in range(2):
                            nc.tensor.matmul(pss[ch][:],
                                             qpT[:, st_g, :],
                                             kpT[:, hl * 2 + j, ch * 512:(ch + 1) * 512],
                                             start=(j == 0), stop=(j == 1))
                    for ch in range(2):
                        nc.vector.tensor_copy(
                            att[hl * 32:hl * 32 + 16, ch * 512:(ch + 1) * 512], pss[ch][:])
                # ---------- E: vp half (PE works while softmax runs on ACT/DVE)
                vp = pKV.tile([128, RT, 1024], bf16, tag="kv")
                for q8 in range(hh * 4, hh * 4 + 4):
                    wvt = pW.tile([128, KT_SD, 256], bf16, tag="wquart")
                    nc.gpsimd.dma_start(wvt[:], wv_d[a, q8])
                    cc = (q8 % 4) * 256
                    for r in range(RT):
                        pv = psum.tile([128, 256], f32, **QUAD)
                        for k in range(KT_SD):
                            nc.tensor.matmul(pv[:], x2T[:, k, r * 128:(r + 1) * 128],
                                             wvt[:, k, :],
                                             start=(k == 0), stop=(k == KT_SD - 1))
                        nc.vector.tensor_copy(vp[:, r, cc:cc + 256], pv[:])
                # ---------- softmax + attT (overlaps E on PE) ----------
                # scores are O(1): exp without max-subtract is safe in f32
                atte = pAtt.tile([128, N], bf16, tag="atte")
                se = pSt.tile([128, 1], f32, tag="st")
                nc.scalar.activation(atte[:], att[:], AF.Exp, accum_out=se[:])
                rs = pSt.tile([128, 1], f32, tag="st")
                nc.vector.reciprocal(rs[:], se[:])
                attn = pAtt.tile([128, N], bf16, tag="attn")
                nc.vector.tensor_scalar_mul(attn[:], atte[:], rs[:])
                attT = pAtt.tile([128, RT, 128], bf16, tag="attT")
                for r in range(RT):
                    pt = psum.tile([128, 128], bf16, name=f"ptt_{a}_{hh}_{r}", **QUAD)
                    nc.tensor.transpose(pt[:], attn[:, r * 128:(r + 1) * 128],
                                        ident[:])
                    nc.vector.tensor_copy(attT[:, r, :], pt[:])
                # ---------- H: ctx half ----------
                pc = [psum.tile([16, 256], f32, name=f"pc_{a}_{hh}_{c}", **QUAD)
                      for c in range(4)]
                for r in range(RT):
                    for hl in range(4):
                        nc.tensor.matmul(pc[hl][:],
                                         attT[:, r, hl * 32:hl * 32 + 16],
                                         vp[:, r, hl * 256:(hl + 1) * 256],
                                         start=(r == 0), stop=(r == RT - 1))
                for c in range(4):
                    nc.vector.tensor_copy(
                        ctx_sb[:, hh * 1024 + c * 256:hh * 1024 + (c + 1) * 256], pc[c][:])

            # ---------- ctxT + I: ctx2 = ctx @ ow (+addq) ----------
            ctxT = pCtx.tile([128, KT_SD, 16], bf16, tag="ctxT")
            for s in range(KT_SD):
                pt = psum.tile([128, 128], bf16, **QUAD)
                nc.tensor.transpose(pt[:, 0:16], ctx_sb[:, s * 128:(s + 1) * 128],
                                    ident[0:16, 0:16])
                nc.vector.tensor_copy(ctxT[:, s, :], pt[:, 0:16])
            addq16 = pOut.tile([16, SD], bf16, tag="addq", bufs=1)
            nc.sync.dma_start(addq16[:], addq_d[a * 16:(a + 1) * 16, :])
            cag = pOut.tile([16, SD], f32, tag="cagg", bufs=2)
            for q8 in range(8):
                owt = pW.tile([128, KT_SD, 256], bf16, tag="wquart")
                nc.gpsimd.dma_start(owt[:], ow_d[a, q8])
                po = psum.tile([16, 256], f32, **QUAD)
                for k in range(KT_SD):
                    nc.tensor.matmul(po[:], ctxT[:, k, :], owt[:, k, :],
                                     start=(k == 0), stop=(k == KT_SD - 1))
                nc.vector.tensor_add(cag[:, q8 * 256:(q8 + 1) * 256],
                                     po[:], addq16[:, q8 * 256:(q8 + 1) * 256])
            # ---------- final LN (per adapter) ----------
            pg16 = pOut.tile([16, SD], bf16, tag="pg", bufs=1)
            nc.sync.dma_start(pg16[:], pg_d[a * 16:(a + 1) * 16, :])
            pb16 = pOut.tile([16, SD], bf16, tag="pb", bufs=1)
            nc.sync.dma_start(pb16[:], pb_d[a * 16:(a + 1) * 16, :])
            s1 = pSt.tile([16, 1], f32, tag="st16")
            nc.vector.tensor_reduce(s1[:], cag[:], axis=mybir.AxisListType.X, op=ALU.add)
            negmu = pSt.tile([16, 1], f32, tag="st16")
            nc.vector.tensor_scalar_mul(negmu[:], s1[:], -1.0 / SD)
            xh = pOut.tile([16, SD], bf16, tag="xh", bufs=1)
            vs = pSt.tile([16, 1], f32, tag="st16")
            nc.scalar.activation(xh[:], cag[:], AF.Square, bias=negmu[:], scale=1.0,
                                 accum_out=vs[:])
            sd_ = pSt.tile([16, 1], f32, tag="st16")
            nc.scalar.activation(sd_[:], vs[:], AF.Sqrt, bias=eps_sb[0:16],
                                 scale=1.0 / SD)
            rstd = pSt.tile([16, 1], f32, tag="st16")
            nc.vector.reciprocal(rstd[:], sd_[:])
            nmr = pSt.tile([16, 1], f32, tag="st16")
            nc.vector.scalar_tensor_tensor(nmr[:], negmu[:], 1.0, rstd[:],
                                           ALU.mult, ALU.mult)
            nc.scalar.activation(xh[:], cag[:], AF.Identity, bias=nmr[:], scale=rstd[:])
            nc.vector.tensor_mul(cag[:], xh[:], pg16[:])
            nc.vector.tensor_add(cag[:], cag[:], pb16[:])
            nc.sync.dma_start(out_d[a * 16:(a + 1) * 16, :], cag[:])

    if split:
        _split_multi_waits(nc)
    return nc


def _bf(x):
    return np.ascontiguousarray(x.astype(ml_dtypes.bfloat16))


def _f32(x):
    return np.ascontiguousarray(x.astype(np.float32))


def _prep_shared(inp):
    def W(nm, a):
        return np.asarray(inp['g_' + nm] if a == 0 else inp['s_' + nm][a - 1],
                          dtype=np.float32)

    dw = np.stack([W('dw', a).reshape(KT_TD, 128, 1024) for a in range(NA)])
    uw = np.stack([W('uw', a).reshape(KT_BD, 128, 2, 1024).transpose(2, 1, 0, 3)
                   for a in range(NA)])  # [a, h, 128, k, 1024]
    wk_l, wv_l, ow_l, wq_l = [], [], [], []
    bq_l, db_l, ub_l, addq_l, pg_l, pb_l, q_l = [], [], [], [], [], [], []
    for a in range(NA):
        lg, lb = W('lg', a), W('lb', a)
        wk_eff = lg[:, None] * W('wk', a)
        wv_eff = lg[:, None] * W('wv', a)
        ow_, ob_, bv_ = W('ow', a), W('ob', a), W('bv', a)
        q_ = W('q', a)
        # [hh, p, part(sd_in), k, col]
        wk_l.append(wk_eff.reshape(KT_SD, 128, 2, 8, 128).transpose(2, 3, 1, 0, 4))
        wv_l.append(wv_eff.reshape(KT_SD, 128, 8, 256).transpose(2, 1, 0, 3))
        ow_l.append(ow_.reshape(KT_SD, 128, 8, 256).transpose(2, 1, 0, 3))
        wq_l.append(W('wq', a).reshape(KT_SD, 128, 8, 256).transpose(2, 1, 0, 3))
        bq_l.append((W('bq', a) / 16.0).reshape(KT_SD, 128).T)
        db_l.append(W('db', a).reshape(KT_BD, 128).T)
        ub_l.append(W('ub', a)[None, :])
        # vp constant term (lb@wv + bv) flows through softmax (rows sum to 1)
        # then ow into the residual; ob adds directly.
        ob_eff = (lb @ W('wv', a) + bv_) @ ow_ + ob_
        addq_l.append(q_ + ob_eff[None, :])
        pg_l.append(np.repeat(W('pg', a)[None, :], K, 0))
        pb_l.append(np.repeat(W('pb', a)[None, :], K, 0))
        q_l.append(q_)
    q_all = np.concatenate(q_l, 0)              # [80, SD]
    qT = (q_all.T / 16.0).reshape(KT_SD, 128, 80).transpose(1, 0, 2)

    return {
        'dw': _bf(dw), 'uw': _bf(uw),
        'wk': _bf(np.stack(wk_l)), 'wv': _bf(np.stack(wv_l)),
        'ow': _bf(np.stack(ow_l)), 'wq': _bf(np.stack(wq_l)),
        'qT': _bf(qT), 'bq': _f32(np.stack(bq_l)), 'db': _f32(np.stack(db_l)),
        'ub': _bf(np.stack(ub_l)), 'addq': _bf(np.concatenate(addq_l, 0)),
        'pg': _bf(np.concatenate(pg_l, 0)), 'pb': _bf(np.concatenate(pb_l, 0)),
    }


def _gelu(x):
    from scipy.special import erf
    return (x * 0.5 * (1.0 + erf(x / np.sqrt(np.float32(2.0))))).astype(np.float32)


def kernel(**inputs):
    if 'nc' not in _CACHE:
        _CACHE['nc'] = _build()
    nc = _CACHE['nc']

    shared = _prep_shared(inputs)
    h = np.asarray(inputs['h_teacher'], dtype=np.float32)
    in_maps = []
    for b in range(B):
        hT = _bf(h[b].T.reshape(KT_TD, 128, 2, 512).transpose(2, 0, 1, 3))
        in_maps.append({**shared, 'hT': hT})

    _CACHE['in_maps'] = in_maps
    res = run_bass_kernel_spmd(nc, in_maps, core_ids=list(range(8)))
    outs = np.stack([res.results[i]['out'] for i in range(B)])  # [8, 80, 2048]

    c_g = outs[:, 0:K, :]
    c_spec = outs[:, K:, :].reshape(B, M, K, SD)

    # host router (fp32, matches reference numerics)
    pooled = h.mean(axis=1)
    r1 = _gelu(pooled @ inputs['r_w1'] + inputs['r_b1'])
    logits = (r1 @ inputs['r_w2'] + inputs['r_b2']).astype(np.float32)
    z = logits - logits.max(-1, keepdims=True)
    ez = np.exp(z)
    probs = (ez / ez.sum(-1, keepdims=True)).astype(np.float32)
    idx = np.argsort(-probs, axis=-1, kind='stable')[:, :TOPK]
    w = np.take_along_axis(probs, idx, axis=-1)
    w = w / (w.sum(-1, keepdims=True) + np.float32(1e-8))
    sel = c_spec[np.arange(B)[:, None], idx]            # [B, TOPK, K, SD]
    c_s = (sel * w[:, :, None, None]).sum(axis=1)
    c_agg = np.concatenate([c_g, c_s], axis=1).astype(np.float32)
    return c_agg, probs.astype(np.float32)
